# revision 3
# baseline (speedup 1.0000x reference)
"""Trainium2 Bass kernel for nn_CMIConnector: visual->ds projection, linear SSM
scan along Lv with time-invariant per-(batch,channel) gates, then out-projection
to d_model. Data-parallel over batch across 8 NeuronCores.

Reference math (per batch row b):
    tc     = mean_Lt(text_embeds[b])                    # [Dt]
    delta  = sigmoid(tc @ Wd.T + bd)                    # [ds]
    B_vec  = tc @ WB.T + bB                             # [ds]
    C_vec  = tc @ WC.T + bC                             # [ds]
    x_t    = visual[b, t] @ Wx.T + bx                   # [ds]
    h_t    = (1-delta) * h_{t-1} + delta*B_vec*x_t      # linear scan over Lv
    out_t  = (C_vec * h_t) @ Wo.T + bo                  # [dm]

The kernel is HBM-bandwidth bound (per core: visual in + d_model out dominate),
so both sides of the big I/O run in fp16: the host pre-casts visual_feats to
fp16 (halving the input read) and the device stores the output in fp16 which
the host upcasts after gather (halving the 128 MiB/core output write). The
rel-err budget (2e-2) dwarfs fp16 rounding (~1e-3 here).

The out-projection runs TRANSPOSED: Wo.T is the stationary PE operand (one
fast-weight-load per 8 matmuls instead of one slow f32 load per matmul) and the
scan output y streams through as the fp16 moving operand; output tiles come out
[d_model-chunk, time] and are stored to a transposed DRAM layout that the host
un-transposes after gather. This also puts the output bias bo on the partition
axis, where the PSUM-evacuation activation adds it for free.
"""

import os
import sys

import numpy as np

for _p in ("/opt/trn_rl_repo",):
    if _p not in sys.path and os.path.isdir(_p):
        sys.path.insert(0, _p)

import concourse.bass as bass  # noqa: E402
import concourse.tile as tile  # noqa: E402
from concourse import bacc, mybir  # noqa: E402
from concourse.bass_utils import run_bass_kernel_spmd  # noqa: E402

F32 = mybir.dt.float32
F32R = mybir.dt.float32r
FP16 = mybir.dt.float16

# Problem shapes (hardcoded per the contract).
B, Lv, Dv = 16, 4096, 1024
Lt, Dt = 128, 4096
DS, DM = 64, 4096
NCORES = 8
BPC = B // NCORES  # batches per core

MM_DTYPE = F32R  # kept for test-harness compat; the big matmuls run fp16

NJ = Dt // 128  # gate contraction chunks
ND = Dv // 128  # x-proj contraction chunks
NT = Lv // 512  # time chunks (x-proj / scan granularity)
NH = 2  # visual halves (DMA granularity: one 4 MiB load per half)
HLEN = Lv // NH
NMC = DM // 128  # out-proj d_model chunks


def _build_program(mm_dtype=MM_DTYPE):
    nc = bacc.Bacc()
    AF = mybir.ActivationFunctionType
    OP = mybir.AluOpType

    # All weight/text tensors are host-packed into their on-chip layouts so
    # every load is one large DMA with contiguous per-partition rows.
    vis16 = nc.dram_tensor("vis16", [BPC, NH, 128, ND * HLEN], FP16, kind="ExternalInput")
    text16 = nc.dram_tensor("text16", [BPC, 128, NJ, Lt], FP16, kind="ExternalInput")
    wxt16 = nc.dram_tensor("wxt16", [128, ND, DS], FP16, kind="ExternalInput")
    wg16 = nc.dram_tensor("wg16", [128, NJ, 3, DS], FP16, kind="ExternalInput")
    wot16 = nc.dram_tensor("wot16", [DS, DM], FP16, kind="ExternalInput")
    bo_t = nc.dram_tensor("bo_t", [128, NMC], F32, kind="ExternalInput")
    bd_c = nc.dram_tensor("bd_c", [DS, 1], F32, kind="ExternalInput")
    nbd_c = nc.dram_tensor("nbd_c", [DS, 1], F32, kind="ExternalInput")
    bb_c = nc.dram_tensor("bb_c", [DS, 1], F32, kind="ExternalInput")
    bc_c = nc.dram_tensor("bc_c", [DS, 1], F32, kind="ExternalInput")
    bx_c = nc.dram_tensor("bx_c", [DS, 1], F32, kind="ExternalInput")
    outT16 = nc.dram_tensor("outT16", [BPC, DM, Lv], FP16, kind="ExternalOutput")

    with tile.TileContext(nc) as tc:
        with (
            tc.tile_pool(name="persist", bufs=1) as persist,
            tc.tile_pool(name="tstream", bufs=2) as tstream,
        ):
            wxt_sb = persist.tile([128, ND, DS], FP16)
            nc.sync.dma_start(out=wxt_sb[:], in_=wxt16[:])
            wg_sb = persist.tile([128, NJ, 3, DS], FP16)
            nc.sync.dma_start(out=wg_sb[:], in_=wg16[:])

            bd_sb = persist.tile([DS, 1], F32)
            nc.sync.dma_start(out=bd_sb[:], in_=bd_c[:])
            nbd_sb = persist.tile([DS, 1], F32)
            nc.sync.dma_start(out=nbd_sb[:], in_=nbd_c[:])
            bb_sb = persist.tile([DS, 1], F32)
            nc.sync.dma_start(out=bb_sb[:], in_=bb_c[:])
            bc_sb = persist.tile([DS, 1], F32)
            nc.sync.dma_start(out=bc_sb[:], in_=bc_c[:])
            bx_sb = persist.tile([DS, 1], F32)
            nc.sync.dma_start(out=bx_sb[:], in_=bx_c[:])

            # ---- Phase 0: fused text-mean gate projections (fp16 PE) ----
            zd_sb = persist.tile([DS, BPC], F32)
            zb_sb = persist.tile([DS, BPC], F32)
            zc_sb = persist.tile([DS, BPC], F32)
            with tc.tile_pool(name="psum0", bufs=2, space="PSUM") as psum0:
                for b in range(BPC):
                    tt = tstream.tile([128, NJ, Lt], FP16, tag="t16")
                    nc.sync.dma_start(out=tt[:], in_=text16[b])
                    zd_ps = psum0.tile([DS, Lt], F32, tag="zd")
                    zb_ps = psum0.tile([DS, Lt], F32, tag="zb")
                    zc_ps = psum0.tile([DS, Lt], F32, tag="zc")
                    for j in range(NJ):
                        for g, ps in enumerate((zd_ps, zb_ps, zc_ps)):
                            nc.tensor.matmul(
                                ps[:],
                                wg_sb[:, j, g, :],
                                tt[:, j, :],
                                start=(j == 0),
                                stop=(j == NJ - 1),
                            )
                    # mean over Lt (1/Lt folded into wg16 on host)
                    nc.vector.reduce_sum(
                        zd_sb[:, b : b + 1], zd_ps[:], axis=mybir.AxisListType.X
                    )
                    nc.vector.reduce_sum(
                        zb_sb[:, b : b + 1], zb_ps[:], axis=mybir.AxisListType.X
                    )
                    nc.vector.reduce_sum(
                        zc_sb[:, b : b + 1], zc_ps[:], axis=mybir.AxisListType.X
                    )

            delta_sb = persist.tile([DS, BPC], F32)
            nc.scalar.activation(
                delta_sb[:], zd_sb[:], AF.Sigmoid, bias=bd_sb[:, 0:1], scale=1.0
            )
            a_sb = persist.tile([DS, BPC], F32)
            nc.scalar.activation(
                a_sb[:], zd_sb[:], AF.Sigmoid, bias=nbd_sb[:, 0:1], scale=-1.0
            )
            bv_sb = persist.tile([DS, BPC], F32)
            nc.vector.tensor_scalar_add(bv_sb[:], zb_sb[:], bb_sb[:, 0:1])
            cv_sb = persist.tile([DS, BPC], F32)
            nc.vector.tensor_scalar_add(cv_sb[:], zc_sb[:], bc_sb[:, 0:1])
            db_sb = persist.tile([DS, BPC], F32)
            nc.vector.tensor_mul(db_sb[:], delta_sb[:], bv_sb[:])
            # Fold the output gate C into the scan input: scanning
            # u'_t = C*delta*B*x_t yields y_t = C*h_t directly.
            cdb_sb = persist.tile([DS, BPC], F32)
            nc.vector.tensor_mul(cdb_sb[:], db_sb[:], cv_sb[:])
            cdbx_sb = persist.tile([DS, BPC], F32)
            nc.vector.tensor_scalar_mul(cdbx_sb[:], cdb_sb[:], bx_sb[:, 0:1])

            # Loaded here (not at the top) so the small gate/x-proj loads win
            # the head of the sync DMA ring and the pipeline starts sooner.
            wot_sb = persist.tile([DS, DM], FP16)
            nc.sync.dma_start(out=wot_sb[:], in_=wot16[:])
            bo_sb = persist.tile([128, NMC], F32)
            nc.sync.dma_start(out=bo_sb[:], in_=bo_t[:])

            # ---- Phases 1+2: x-proj + chunked scan, then out-proj ----
            evac_i = [0]

            with (
                tc.tile_pool(name="psx", bufs=2, space="PSUM") as psx,
                tc.tile_pool(name="pso", bufs=3, space="PSUM") as pso,
                tc.tile_pool(name="visb", bufs=2) as visb,
                tc.tile_pool(name="ubp", bufs=2) as ubp,
                tc.tile_pool(name="abp", bufs=2) as abp,
                tc.tile_pool(name="outp", bufs=3) as outp,
            ):
                ys = {}

                def phase_a(b):
                    u_t = ubp.tile([DS, Lv], F32, tag="u")
                    y_r = ubp.tile([DS, Lv], FP16, tag="y")
                    ys[b] = y_r
                    # per-chunk broadcast of the decay gate a=(1-delta): the
                    # scan consumes the same [DS, 512] columns every chunk.
                    a_bc = abp.tile([DS, 512], F32, tag="a")
                    nc.gpsimd.memset(a_bc[:], 1.0)
                    nc.vector.tensor_scalar_mul(a_bc[:], a_bc[:], a_sb[:, b : b + 1])

                    vis_tiles = {}
                    for h in range(NH):
                        vt = visb.tile([128, ND * HLEN], FP16, tag="v")
                        nc.sync.dma_start(out=vt[:], in_=vis16[b, h])
                        vis_tiles[h] = vt

                    for t in range(NT):
                        sl = slice(t * 512, (t + 1) * 512)
                        h, i = divmod(t, NT // NH)
                        vt = vis_tiles[h]
                        xp = psx.tile([DS, 512], F32, tag="x")
                        for d in range(ND):
                            nc.tensor.matmul(
                                xp[:],
                                wxt_sb[:, d, :],
                                vt[:, d * HLEN + i * 512 : d * HLEN + (i + 1) * 512],
                                start=(d == 0),
                                stop=(d == ND - 1),
                            )
                        # u = (C*deltaB) * x_raw + (C*deltaB)*bx
                        nc.scalar.activation(
                            u_t[:, sl],
                            xp[:],
                            AF.Identity,
                            bias=cdbx_sb[:, b : b + 1],
                            scale=cdb_sb[:, b : b + 1],
                        )
                        # chunked scan; chain via the previous chunk's last col
                        nc.vector.tensor_tensor_scan(
                            y_r[:, sl],
                            a_bc[:],
                            u_t[:, sl],
                            0.0 if t == 0 else y_r[:, t * 512 - 1 : t * 512],
                            OP.mult,
                            OP.add,
                        )

                def phase_b(b):
                    y_r = ys[b]
                    for mc in range(NMC):
                        ot = outp.tile([128, Lv], FP16, tag="o")
                        lhs = wot_sb[:, mc * 128 : (mc + 1) * 128]
                        for q in range(Lv // 1024):
                            op_ = pso.tile([128, 1024], F32, tag="op")
                            for hh in range(2):
                                nc.tensor.matmul(
                                    op_[:, hh * 512 : (hh + 1) * 512],
                                    lhs,
                                    y_r[:, q * 1024 + hh * 512 : q * 1024 + (hh + 1) * 512],
                                    start=True,
                                    stop=True,
                                )
                            dst = ot[:, q * 1024 : (q + 1) * 1024]
                            # PSUM evacuation (f32 -> fp16, + bias bo on the
                            # partition axis), alternated across the only two
                            # engines with a PSUM read port.
                            if evac_i[0] % 2 == 0:
                                nc.scalar.activation(
                                    dst,
                                    op_[:],
                                    AF.Identity,
                                    bias=bo_sb[:, mc : mc + 1],
                                    scale=1.0,
                                )
                            else:
                                nc.vector.tensor_scalar_add(
                                    dst, op_[:], bo_sb[:, mc : mc + 1]
                                )
                            evac_i[0] += 1
                        nc.scalar.dma_start(
                            out=outT16[b, mc * 128 : (mc + 1) * 128, :], in_=ot[:]
                        )

                # A0 A1 B0 B1: both scans run before the first out-proj batch
                # so no engine's program order interleaves scan work behind the
                # long evacuation stream (the scan chain would stall phase B).
                for b in range(BPC):
                    phase_a(b)
                for b in range(BPC):
                    phase_b(b)
    return nc


def _prep_host_inputs(
    visual_feats, text_embeds, Wx, bx, Wd, bd, WB, bB, WC, bC, Wo, bo
):
    f = lambda a: np.asarray(a, dtype=np.float32)
    # [B, Lv, Dv] -> [B, NH, 128p, ND*HLEN] fp16 with element
    # (b, h, p, d*HLEN+t) = visual[b, h*HLEN+t, d*128+p]
    vis16 = np.ascontiguousarray(
        f(visual_feats)
        .transpose(0, 2, 1)
        .reshape(B, ND, 128, NH, HLEN)
        .transpose(0, 3, 2, 1, 4)
        .reshape(B, NH, 128, ND * HLEN)
        .astype(np.float16)
    )
    # [B, Lt, Dt] -> [B, 128p, NJ, Lt] with Dt index = j*128 + p
    text16 = np.ascontiguousarray(
        f(text_embeds)
        .transpose(0, 2, 1)
        .reshape(B, NJ, 128, Lt)
        .transpose(0, 2, 1, 3)
        .astype(np.float16)
    )
    # Wx.T [Dv, ds] -> [128p, ND, ds] with Dv index = c*128 + p
    wxt16 = np.ascontiguousarray(
        f(Wx).T.reshape(ND, 128, DS).transpose(1, 0, 2).astype(np.float16)
    )
    # Gate weights transposed, pre-scaled by 1/Lt (the text mean), fp16,
    # packed [Dt, 3, ds] -> [128p, NJ, 3, ds] with Dt index = j*128 + p.
    wg16 = np.ascontiguousarray(
        (np.stack([f(Wd).T, f(WB).T, f(WC).T], axis=1) / np.float32(Lt))
        .reshape(NJ, 128, 3, DS)
        .transpose(1, 0, 2, 3)
        .astype(np.float16)
    )
    wot16 = np.ascontiguousarray(f(Wo).T.astype(np.float16))  # [ds, dm]
    # bo -> [128p, NMC] with dm index = mc*128 + p
    bo_t = np.ascontiguousarray(f(bo).reshape(NMC, 128).T)
    col = lambda a: np.ascontiguousarray(f(a).reshape(-1, 1))
    shared = {
        "wxt16": wxt16,
        "wg16": wg16,
        "wot16": wot16,
        "bo_t": bo_t,
        "bd_c": col(bd),
        "nbd_c": col(-f(bd)),
        "bb_c": col(bB),
        "bc_c": col(bC),
        "bx_c": col(bx),
    }
    in_maps = []
    for c in range(NCORES):
        m = dict(shared)
        m["vis16"] = np.ascontiguousarray(vis16[c * BPC : (c + 1) * BPC])
        m["text16"] = np.ascontiguousarray(text16[c * BPC : (c + 1) * BPC])
        in_maps.append(m)
    return in_maps


_PROGRAM_CACHE = {}


def _get_program(mm_dtype=MM_DTYPE):
    key = str(mm_dtype)
    if key not in _PROGRAM_CACHE:
        nc = _build_program(mm_dtype)
        if not nc.is_finalized():
            nc.finalize()
        _PROGRAM_CACHE[key] = nc
    return _PROGRAM_CACHE[key]


def run(inputs: dict, trace: bool = False, mm_dtype=MM_DTYPE):
    """Run the kernel on all 8 cores; returns (full_output, BassKernelResults)."""
    nc = _get_program(mm_dtype)
    in_maps = _prep_host_inputs(**inputs)
    res = run_bass_kernel_spmd(nc, in_maps, list(range(NCORES)), trace=trace)
    # outT16 is [BPC, DM, Lv] fp16 per core; un-transpose + upcast on host.
    full = np.concatenate(
        [
            np.swapaxes(res.results[i]["outT16"], 1, 2).astype(np.float32)
            for i in range(NCORES)
        ],
        axis=0,
    )
    return np.ascontiguousarray(full), res


def kernel(**inputs) -> np.ndarray:
    out, _ = run(inputs, trace=False)
    return out


# revision 6
# speedup vs baseline: 1.3508x; 1.3508x over previous
"""Trainium2 Bass kernel for nn_CMIConnector: visual->ds projection, linear SSM
scan along Lv with time-invariant per-(batch,channel) gates, then out-projection
to d_model. Data-parallel over batch across 8 NeuronCores.

Reference math (per batch row b):
    tc     = mean_Lt(text_embeds[b])                    # [Dt]
    delta  = sigmoid(tc @ Wd.T + bd)                    # [ds]
    B_vec  = tc @ WB.T + bB                             # [ds]
    C_vec  = tc @ WC.T + bC                             # [ds]
    x_t    = visual[b, t] @ Wx.T + bx                   # [ds]
    h_t    = (1-delta) * h_{t-1} + delta*B_vec*x_t      # linear scan over Lv
    out_t  = (C_vec * h_t) @ Wo.T + bo                  # [dm]

The kernel is HBM-bandwidth bound (per core: visual in + d_model out dominate),
so both sides of the big I/O run in fp16: the host pre-casts visual_feats to
fp16 (halving the input read) and the device stores the output in fp16 which
the host upcasts after gather (halving the 128 MiB/core output write). The
rel-err budget (2e-2) dwarfs fp16 rounding (~1e-3 here).

The out-projection runs TRANSPOSED: Wo.T is the stationary PE operand and the
scan output y streams through as the fp16 moving operand; output tiles come out
[d_model-chunk, time] and are stored to a transposed DRAM layout that the host
un-transposes after gather. This puts the output bias bo on the partition axis,
where the PSUM-evacuation activation adds it for free.

Schedule: work is split into half-Lv windows, x-proj+scan (A) running two
windows ahead of the out-proj+store (B) stream it feeds:
    gates  A00 A01  B00  A10  B01  A11  B10  B11
so output stores saturate the DMA from ~20us onward, the scan of window k+1
lands on the vector engine before window k's long evacuation stream (no carry
stall), and the PE instruction stream stays dense (a saturated PE un-throttles
the HAM clock gate; a 50%-duty PE with ~1.4us gaps runs at half clock forever
- measured on v3 of this kernel). PSUM evacuation (the only PSUM->SBUF path
is ScalarE/VectorE) is split 2:1 scalar:vector per their measured rates, and
output stores issue on the Sync HWDGE ring while input loads issue on the
Scalar ring so a 4 MiB visual load never delays the store stream.
"""

import os
import sys

import numpy as np

for _p in ("/opt/trn_rl_repo",):
    if _p not in sys.path and os.path.isdir(_p):
        sys.path.insert(0, _p)

import concourse.bass as bass  # noqa: E402
import concourse.tile as tile  # noqa: E402
from concourse import bacc, mybir  # noqa: E402
from concourse.bass_utils import run_bass_kernel_spmd  # noqa: E402

F32 = mybir.dt.float32
F32R = mybir.dt.float32r
FP16 = mybir.dt.float16

# Problem shapes (hardcoded per the contract).
B, Lv, Dv = 16, 4096, 1024
Lt, Dt = 128, 4096
DS, DM = 64, 4096
NCORES = 8
BPC = B // NCORES  # batches per core

MM_DTYPE = F32R  # kept for test-harness compat; the big matmuls run fp16

NJ = Dt // 128  # gate contraction chunks
ND = Dv // 128  # x-proj contraction chunks
NH = 2  # halves of Lv (pipeline window = one half)
HLEN = Lv // NH
NTH = HLEN // 512  # 512-wide time chunks per half
NMC = DM // 128  # out-proj d_model chunks


def _build_program(mm_dtype=MM_DTYPE):
    nc = bacc.Bacc()
    AF = mybir.ActivationFunctionType
    OP = mybir.AluOpType

    # All weight/text tensors are host-packed into their on-chip layouts so
    # every load is one large DMA with contiguous per-partition rows.
    vis16 = nc.dram_tensor("vis16", [BPC, NH, 128, ND * HLEN], FP16, kind="ExternalInput")
    text16 = nc.dram_tensor("text16", [BPC, 128, NJ, Lt], FP16, kind="ExternalInput")
    wxt16 = nc.dram_tensor("wxt16", [128, ND, DS], FP16, kind="ExternalInput")
    wg16 = nc.dram_tensor("wg16", [128, NJ, 3, DS], FP16, kind="ExternalInput")
    wot16 = nc.dram_tensor("wot16", [DS, DM], FP16, kind="ExternalInput")
    bo_t = nc.dram_tensor("bo_t", [128, NMC], F32, kind="ExternalInput")
    bd_c = nc.dram_tensor("bd_c", [DS, 1], F32, kind="ExternalInput")
    nbd_c = nc.dram_tensor("nbd_c", [DS, 1], F32, kind="ExternalInput")
    bb_c = nc.dram_tensor("bb_c", [DS, 1], F32, kind="ExternalInput")
    bc_c = nc.dram_tensor("bc_c", [DS, 1], F32, kind="ExternalInput")
    bx_c = nc.dram_tensor("bx_c", [DS, 1], F32, kind="ExternalInput")
    outT16 = nc.dram_tensor("outT16", [BPC, DM, Lv], FP16, kind="ExternalOutput")

    with tile.TileContext(nc) as tc:
        with (
            tc.tile_pool(name="persist", bufs=1) as persist,
            tc.tile_pool(name="tstream", bufs=2) as tstream,
        ):
            wxt_sb = persist.tile([128, ND, DS], FP16)
            nc.scalar.dma_start(out=wxt_sb[:], in_=wxt16[:])
            wg_sb = persist.tile([128, NJ, 3, DS], FP16)
            nc.scalar.dma_start(out=wg_sb[:], in_=wg16[:])

            bd_sb = persist.tile([DS, 1], F32)
            nc.scalar.dma_start(out=bd_sb[:], in_=bd_c[:])
            nbd_sb = persist.tile([DS, 1], F32)
            nc.scalar.dma_start(out=nbd_sb[:], in_=nbd_c[:])
            bb_sb = persist.tile([DS, 1], F32)
            nc.scalar.dma_start(out=bb_sb[:], in_=bb_c[:])
            bc_sb = persist.tile([DS, 1], F32)
            nc.scalar.dma_start(out=bc_sb[:], in_=bc_c[:])
            bx_sb = persist.tile([DS, 1], F32)
            nc.scalar.dma_start(out=bx_sb[:], in_=bx_c[:])

            # ---- Phase 0: fused text-mean gate projections (fp16 PE) ----
            zd_sb = persist.tile([DS, BPC], F32)
            zb_sb = persist.tile([DS, BPC], F32)
            zc_sb = persist.tile([DS, BPC], F32)
            with tc.tile_pool(name="psum0", bufs=2, space="PSUM") as psum0:
                for b in range(BPC):
                    tt = tstream.tile([128, NJ, Lt], FP16, tag="t16")
                    nc.scalar.dma_start(out=tt[:], in_=text16[b])
                    zd_ps = psum0.tile([DS, Lt], F32, tag="zd")
                    zb_ps = psum0.tile([DS, Lt], F32, tag="zb")
                    zc_ps = psum0.tile([DS, Lt], F32, tag="zc")
                    for j in range(NJ):
                        for g, ps in enumerate((zd_ps, zb_ps, zc_ps)):
                            nc.tensor.matmul(
                                ps[:],
                                wg_sb[:, j, g, :],
                                tt[:, j, :],
                                start=(j == 0),
                                stop=(j == NJ - 1),
                            )
                    # mean over Lt (1/Lt folded into wg16 on host)
                    nc.vector.reduce_sum(
                        zd_sb[:, b : b + 1], zd_ps[:], axis=mybir.AxisListType.X
                    )
                    nc.vector.reduce_sum(
                        zb_sb[:, b : b + 1], zb_ps[:], axis=mybir.AxisListType.X
                    )
                    nc.vector.reduce_sum(
                        zc_sb[:, b : b + 1], zc_ps[:], axis=mybir.AxisListType.X
                    )

            delta_sb = persist.tile([DS, BPC], F32)
            nc.scalar.activation(
                delta_sb[:], zd_sb[:], AF.Sigmoid, bias=bd_sb[:, 0:1], scale=1.0
            )
            a_sb = persist.tile([DS, BPC], F32)
            nc.scalar.activation(
                a_sb[:], zd_sb[:], AF.Sigmoid, bias=nbd_sb[:, 0:1], scale=-1.0
            )
            bv_sb = persist.tile([DS, BPC], F32)
            nc.vector.tensor_scalar_add(bv_sb[:], zb_sb[:], bb_sb[:, 0:1])
            cv_sb = persist.tile([DS, BPC], F32)
            nc.vector.tensor_scalar_add(cv_sb[:], zc_sb[:], bc_sb[:, 0:1])
            db_sb = persist.tile([DS, BPC], F32)
            nc.vector.tensor_mul(db_sb[:], delta_sb[:], bv_sb[:])
            # Fold the output gate C into the scan input: scanning
            # u'_t = C*delta*B*x_t yields y_t = C*h_t directly.
            cdb_sb = persist.tile([DS, BPC], F32)
            nc.vector.tensor_mul(cdb_sb[:], db_sb[:], cv_sb[:])
            cdbx_sb = persist.tile([DS, BPC], F32)
            nc.vector.tensor_scalar_mul(cdbx_sb[:], cdb_sb[:], bx_sb[:, 0:1])

            wot_sb = persist.tile([DS, DM], FP16)
            nc.scalar.dma_start(out=wot_sb[:], in_=wot16[:])
            bo_sb = persist.tile([128, NMC], F32)
            nc.scalar.dma_start(out=bo_sb[:], in_=bo_t[:])

            # ---- Phases 1+2: x-proj + chunked scan (A), out-proj (B) ----
            evac_i = [0]

            with (
                tc.tile_pool(name="psx", bufs=2, space="PSUM") as psx,
                tc.tile_pool(name="pso", bufs=3, space="PSUM") as pso,
                tc.tile_pool(name="visb", bufs=3) as visb,
                tc.tile_pool(name="ubp", bufs=2) as ubp,
                tc.tile_pool(name="abp", bufs=2) as abp,
                tc.tile_pool(name="outp", bufs=3) as outp,
            ):
                ys, us, abcs, vts = {}, {}, {}, {}

                def load_vis(b, h):
                    vt = visb.tile([128, ND * HLEN], FP16, tag="v", name="vt")
                    nc.scalar.dma_start(out=vt[:], in_=vis16[b, h])
                    vts[(b, h)] = vt

                def phase_a(b, h):
                    if h == 0:
                        us[b] = ubp.tile([DS, Lv], F32, tag="u", name="u_t")
                        ys[b] = ubp.tile([DS, Lv], FP16, tag="y", name="y_r")
                        # broadcast decay gate a=(1-delta); the scan consumes
                        # the same [DS, 512] columns every chunk.
                        a_bc = abp.tile([DS, 512], F32, tag="a", name="a_bc")
                        nc.gpsimd.memset(a_bc[:], 1.0)
                        nc.vector.tensor_scalar_mul(
                            a_bc[:], a_bc[:], a_sb[:, b : b + 1]
                        )
                        abcs[b] = a_bc
                    u_t, y_r, a_bc = us[b], ys[b], abcs[b]
                    vt = vts[(b, h)]
                    for i in range(NTH):
                        t = h * NTH + i
                        sl = slice(t * 512, (t + 1) * 512)
                        xp = psx.tile([DS, 512], F32, tag="x")
                        for d in range(ND):
                            nc.tensor.matmul(
                                xp[:],
                                wxt_sb[:, d, :],
                                vt[:, d * HLEN + i * 512 : d * HLEN + (i + 1) * 512],
                                start=(d == 0),
                                stop=(d == ND - 1),
                            )
                        # u = (C*deltaB) * x_raw + (C*deltaB)*bx
                        nc.scalar.activation(
                            u_t[:, sl],
                            xp[:],
                            AF.Identity,
                            bias=cdbx_sb[:, b : b + 1],
                            scale=cdb_sb[:, b : b + 1],
                        )
                        # chunked scan; chain via the previous chunk's last col
                        nc.vector.tensor_tensor_scan(
                            y_r[:, sl],
                            a_bc[:],
                            u_t[:, sl],
                            0.0 if t == 0 else y_r[:, t * 512 - 1 : t * 512],
                            OP.mult,
                            OP.add,
                        )

                def phase_b(b, h):
                    y_r = ys[b]
                    for mc in range(NMC):
                        ot = outp.tile([128, HLEN], FP16, tag="o")
                        lhs = wot_sb[:, mc * 128 : (mc + 1) * 128]
                        for q in range(HLEN // 1024):
                            base = h * HLEN + q * 1024
                            op_ = pso.tile([128, 1024], F32, tag="op")
                            for hh in range(2):
                                nc.tensor.matmul(
                                    op_[:, hh * 512 : (hh + 1) * 512],
                                    lhs,
                                    y_r[:, base + hh * 512 : base + (hh + 1) * 512],
                                    start=True,
                                    stop=True,
                                )
                            dst = ot[:, q * 1024 : (q + 1) * 1024]
                            # PSUM evacuation (f32->fp16 + bias bo), split 2:1
                            # scalar:vector per measured engine rates.
                            if evac_i[0] % 3 < 2:
                                nc.scalar.activation(
                                    dst,
                                    op_[:],
                                    AF.Identity,
                                    bias=bo_sb[:, mc : mc + 1],
                                    scale=1.0,
                                )
                            else:
                                nc.vector.tensor_scalar_add(
                                    dst, op_[:], bo_sb[:, mc : mc + 1]
                                )
                            evac_i[0] += 1
                        nc.sync.dma_start(
                            out=outT16[
                                b, mc * 128 : (mc + 1) * 128, h * HLEN : (h + 1) * HLEN
                            ],
                            in_=ot[:],
                        )

                load_vis(0, 0)
                load_vis(0, 1)
                phase_a(0, 0)
                load_vis(1, 0)
                phase_a(0, 1)
                phase_b(0, 0)
                load_vis(1, 1)
                phase_a(1, 0)
                phase_b(0, 1)
                phase_a(1, 1)
                phase_b(1, 0)
                phase_b(1, 1)
    return nc


def _prep_host_inputs(
    visual_feats, text_embeds, Wx, bx, Wd, bd, WB, bB, WC, bC, Wo, bo
):
    f = lambda a: np.asarray(a, dtype=np.float32)
    # [B, Lv, Dv] -> [B, NH, 128p, ND*HLEN] fp16 with element
    # (b, h, p, d*HLEN+t) = visual[b, h*HLEN+t, d*128+p]
    vis16 = np.ascontiguousarray(
        f(visual_feats)
        .transpose(0, 2, 1)
        .reshape(B, ND, 128, NH, HLEN)
        .transpose(0, 3, 2, 1, 4)
        .reshape(B, NH, 128, ND * HLEN)
        .astype(np.float16)
    )
    # [B, Lt, Dt] -> [B, 128p, NJ, Lt] with Dt index = j*128 + p
    text16 = np.ascontiguousarray(
        f(text_embeds)
        .transpose(0, 2, 1)
        .reshape(B, NJ, 128, Lt)
        .transpose(0, 2, 1, 3)
        .astype(np.float16)
    )
    # Wx.T [Dv, ds] -> [128p, ND, ds] with Dv index = c*128 + p
    wxt16 = np.ascontiguousarray(
        f(Wx).T.reshape(ND, 128, DS).transpose(1, 0, 2).astype(np.float16)
    )
    # Gate weights transposed, pre-scaled by 1/Lt (the text mean), fp16,
    # packed [Dt, 3, ds] -> [128p, NJ, 3, ds] with Dt index = j*128 + p.
    wg16 = np.ascontiguousarray(
        (np.stack([f(Wd).T, f(WB).T, f(WC).T], axis=1) / np.float32(Lt))
        .reshape(NJ, 128, 3, DS)
        .transpose(1, 0, 2, 3)
        .astype(np.float16)
    )
    wot16 = np.ascontiguousarray(f(Wo).T.astype(np.float16))  # [ds, dm]
    # bo -> [128p, NMC] with dm index = mc*128 + p
    bo_t = np.ascontiguousarray(f(bo).reshape(NMC, 128).T)
    col = lambda a: np.ascontiguousarray(f(a).reshape(-1, 1))
    shared = {
        "wxt16": wxt16,
        "wg16": wg16,
        "wot16": wot16,
        "bo_t": bo_t,
        "bd_c": col(bd),
        "nbd_c": col(-f(bd)),
        "bb_c": col(bB),
        "bc_c": col(bC),
        "bx_c": col(bx),
    }
    in_maps = []
    for c in range(NCORES):
        m = dict(shared)
        m["vis16"] = np.ascontiguousarray(vis16[c * BPC : (c + 1) * BPC])
        m["text16"] = np.ascontiguousarray(text16[c * BPC : (c + 1) * BPC])
        in_maps.append(m)
    return in_maps


_PROGRAM_CACHE = {}


def _get_program(mm_dtype=MM_DTYPE):
    key = str(mm_dtype)
    if key not in _PROGRAM_CACHE:
        nc = _build_program(mm_dtype)
        if not nc.is_finalized():
            nc.finalize()
        _PROGRAM_CACHE[key] = nc
    return _PROGRAM_CACHE[key]


def run(inputs: dict, trace: bool = False, mm_dtype=MM_DTYPE):
    """Run the kernel on all 8 cores; returns (full_output, BassKernelResults)."""
    nc = _get_program(mm_dtype)
    in_maps = _prep_host_inputs(**inputs)
    res = run_bass_kernel_spmd(nc, in_maps, list(range(NCORES)), trace=trace)
    # outT16 is [BPC, DM, Lv] fp16 per core; un-transpose + upcast on host.
    full = np.concatenate(
        [
            np.swapaxes(res.results[i]["outT16"], 1, 2).astype(np.float32)
            for i in range(NCORES)
        ],
        axis=0,
    )
    return np.ascontiguousarray(full), res


def kernel(**inputs) -> np.ndarray:
    out, _ = run(inputs, trace=False)
    return out


# revision 7
# speedup vs baseline: 1.4467x; 1.0710x over previous
"""Trainium2 Bass kernel for nn_CMIConnector: visual->ds projection, linear SSM
scan along Lv with time-invariant per-(batch,channel) gates, then out-projection
to d_model. Data-parallel over batch across 8 NeuronCores.

Reference math (per batch row b):
    tc     = mean_Lt(text_embeds[b])                    # [Dt]
    delta  = sigmoid(tc @ Wd.T + bd)                    # [ds]
    B_vec  = tc @ WB.T + bB                             # [ds]
    C_vec  = tc @ WC.T + bC                             # [ds]
    x_t    = visual[b, t] @ Wx.T + bx                   # [ds]
    h_t    = (1-delta) * h_{t-1} + delta*B_vec*x_t      # linear scan over Lv
    out_t  = (C_vec * h_t) @ Wo.T + bo                  # [dm]

The kernel is HBM-bandwidth bound (per core: visual in + d_model out dominate),
so both sides of the big I/O run in fp16: the host pre-casts visual_feats to
fp16 and the device stores fp16 output which the host upcasts after gather.
The rel-err budget (2e-2) dwarfs fp16 rounding (~1e-3 here).

The out-projection runs TRANSPOSED (Wo.T stationary, scan output y as the
fp16 moving operand); output tiles come out [d_model-chunk, time] and go to a
transposed DRAM layout the host un-transposes. The bias bo lands on the
partition axis where PSUM evacuation adds it for free.

Everything ds-sized is DUPLICATED onto both 64-partition halves: the gate and
x-proj matmuls issue column-tiled twins ((0,0)/(0,64) — concurrent in the PE
array), the scan runs on all 128 partitions (the DVE is 128-lane, so the copy
is free), and the out-projection then row-packs TWO d_model chunks as
tile-position (0,0)/(64,0) pairs that execute concurrently. That halves PE
time per output tile, which makes the kernel's pace independent of the HAM
clock gate: this instruction mix idles the PE a little every ~1us, which keeps
the PE throttled at 1.2 GHz essentially forever (measured on earlier versions
- a 94%-busy PE never saw one un-throttled window), so the design assumes the
COLD clock and keeps the PE off the critical path anyway.

Schedule: half-Lv windows, x-proj+scan (A) running two windows ahead of the
out-proj+store (B) stream it feeds:
    gates  A00 A01  B00  A10  B01  A11  B10  B11
so output stores saturate DMA from ~50us onward and the scan of window k+1
lands on the vector engine before window k's evacuation stream. PSUM
evacuation (ScalarE/VectorE are the only PSUM readers) splits 2:1
scalar:vector per measured rates; stores issue on the Sync HWDGE ring, loads
on the Scalar ring, so a 4 MiB visual load never delays the store stream.
"""

import os
import sys

import numpy as np

for _p in ("/opt/trn_rl_repo",):
    if _p not in sys.path and os.path.isdir(_p):
        sys.path.insert(0, _p)

import concourse.bass as bass  # noqa: E402
import concourse.tile as tile  # noqa: E402
from concourse import bacc, mybir  # noqa: E402
from concourse.bass_utils import run_bass_kernel_spmd  # noqa: E402

F32 = mybir.dt.float32
F32R = mybir.dt.float32r
FP16 = mybir.dt.float16

# Problem shapes (hardcoded per the contract).
B, Lv, Dv = 16, 4096, 1024
Lt, Dt = 128, 4096
DS, DM = 64, 4096
NCORES = 8
BPC = B // NCORES  # batches per core

MM_DTYPE = F32R  # kept for test-harness compat; the big matmuls run fp16

NJ = Dt // 128  # gate contraction chunks
ND = Dv // 128  # x-proj contraction chunks
NH = 2  # halves of Lv (pipeline window = one half)
HLEN = Lv // NH
NTH = HLEN // 512  # 512-wide time chunks per half
NMC = DM // 128  # out-proj d_model chunks


def _build_program(mm_dtype=MM_DTYPE):
    nc = bacc.Bacc()
    AF = mybir.ActivationFunctionType
    OP = mybir.AluOpType

    # All weight/text tensors are host-packed into their on-chip layouts so
    # every load is one large DMA with contiguous per-partition rows.
    vis16 = nc.dram_tensor("vis16", [BPC, NH, 128, ND * HLEN], FP16, kind="ExternalInput")
    text16 = nc.dram_tensor("text16", [BPC, 128, NJ, Lt], FP16, kind="ExternalInput")
    wxt16 = nc.dram_tensor("wxt16", [128, ND, DS], FP16, kind="ExternalInput")
    wg16 = nc.dram_tensor("wg16", [128, NJ, 3, DS], FP16, kind="ExternalInput")
    wott16 = nc.dram_tensor("wott16", [128, DM], FP16, kind="ExternalInput")
    bo_t = nc.dram_tensor("bo_t", [128, NMC], F32, kind="ExternalInput")
    # per-ds bias columns, duplicated onto both 64-partition halves
    bd_c = nc.dram_tensor("bd_c", [128, 1], F32, kind="ExternalInput")
    nbd_c = nc.dram_tensor("nbd_c", [128, 1], F32, kind="ExternalInput")
    bb_c = nc.dram_tensor("bb_c", [128, 1], F32, kind="ExternalInput")
    bc_c = nc.dram_tensor("bc_c", [128, 1], F32, kind="ExternalInput")
    bx_c = nc.dram_tensor("bx_c", [128, 1], F32, kind="ExternalInput")
    outT16 = nc.dram_tensor("outT16", [BPC, DM, Lv], FP16, kind="ExternalOutput")

    with tile.TileContext(nc) as tc:
        with (
            tc.tile_pool(name="persist", bufs=1) as persist,
            tc.tile_pool(name="tstream", bufs=2) as tstream,
        ):
            wxt_sb = persist.tile([128, ND, DS], FP16)
            nc.scalar.dma_start(out=wxt_sb[:], in_=wxt16[:])
            wg_sb = persist.tile([128, NJ, 3, DS], FP16)
            nc.scalar.dma_start(out=wg_sb[:], in_=wg16[:])

            bd_sb = persist.tile([128, 1], F32)
            nc.scalar.dma_start(out=bd_sb[:], in_=bd_c[:])
            nbd_sb = persist.tile([128, 1], F32)
            nc.scalar.dma_start(out=nbd_sb[:], in_=nbd_c[:])
            bb_sb = persist.tile([128, 1], F32)
            nc.scalar.dma_start(out=bb_sb[:], in_=bb_c[:])
            bc_sb = persist.tile([128, 1], F32)
            nc.scalar.dma_start(out=bc_sb[:], in_=bc_c[:])
            bx_sb = persist.tile([128, 1], F32)
            nc.scalar.dma_start(out=bx_sb[:], in_=bx_c[:])

            # ---- Phase 0: fused text-mean gate projections (fp16 PE) ----
            # Every z is computed twice via column-tiled twin matmuls so the
            # whole gate chain lives on all 128 partitions (rows 64-127 are a
            # copy of 0-63), which the scan and out-proj packing rely on.
            zd_sb = persist.tile([128, BPC], F32)
            zb_sb = persist.tile([128, BPC], F32)
            zc_sb = persist.tile([128, BPC], F32)
            with tc.tile_pool(name="psum0", bufs=2, space="PSUM") as psum0:
                for b in range(BPC):
                    tt = tstream.tile([128, NJ, Lt], FP16, tag="t16")
                    nc.scalar.dma_start(out=tt[:], in_=text16[b])
                    zd_ps = psum0.tile([128, Lt], F32, tag="zd")
                    zb_ps = psum0.tile([128, Lt], F32, tag="zb")
                    zc_ps = psum0.tile([128, Lt], F32, tag="zc")
                    for j in range(NJ):
                        for g, ps in enumerate((zd_ps, zb_ps, zc_ps)):
                            for lo in (0, DS):
                                nc.tensor.matmul(
                                    ps[lo : lo + DS, :],
                                    wg_sb[:, j, g, :],
                                    tt[:, j, :],
                                    start=(j == 0),
                                    stop=(j == NJ - 1),
                                )
                    # mean over Lt (1/Lt folded into wg16 on host)
                    nc.vector.reduce_sum(
                        zd_sb[:, b : b + 1], zd_ps[:], axis=mybir.AxisListType.X
                    )
                    nc.vector.reduce_sum(
                        zb_sb[:, b : b + 1], zb_ps[:], axis=mybir.AxisListType.X
                    )
                    nc.vector.reduce_sum(
                        zc_sb[:, b : b + 1], zc_ps[:], axis=mybir.AxisListType.X
                    )

            delta_sb = persist.tile([128, BPC], F32)
            nc.scalar.activation(
                delta_sb[:], zd_sb[:], AF.Sigmoid, bias=bd_sb[:, 0:1], scale=1.0
            )
            a_sb = persist.tile([128, BPC], F32)
            nc.scalar.activation(
                a_sb[:], zd_sb[:], AF.Sigmoid, bias=nbd_sb[:, 0:1], scale=-1.0
            )
            bv_sb = persist.tile([128, BPC], F32)
            nc.vector.tensor_scalar_add(bv_sb[:], zb_sb[:], bb_sb[:, 0:1])
            cv_sb = persist.tile([128, BPC], F32)
            nc.vector.tensor_scalar_add(cv_sb[:], zc_sb[:], bc_sb[:, 0:1])
            db_sb = persist.tile([128, BPC], F32)
            nc.vector.tensor_mul(db_sb[:], delta_sb[:], bv_sb[:])
            # Fold the output gate C into the scan input: scanning
            # u'_t = C*delta*B*x_t yields y_t = C*h_t directly.
            cdb_sb = persist.tile([128, BPC], F32)
            nc.vector.tensor_mul(cdb_sb[:], db_sb[:], cv_sb[:])
            cdbx_sb = persist.tile([128, BPC], F32)
            nc.vector.tensor_scalar_mul(cdbx_sb[:], cdb_sb[:], bx_sb[:, 0:1])

            wott_sb = persist.tile([128, DM], FP16)
            nc.scalar.dma_start(out=wott_sb[:], in_=wott16[:])
            bo_sb = persist.tile([128, NMC], F32)
            nc.scalar.dma_start(out=bo_sb[:], in_=bo_t[:])

            # ---- Phases 1+2: x-proj + chunked scan (A), out-proj (B) ----
            evac_i = [0]

            with (
                tc.tile_pool(name="psx", bufs=2, space="PSUM") as psx,
                tc.tile_pool(name="pso", bufs=3, space="PSUM") as pso,
                tc.tile_pool(name="visb", bufs=3) as visb,
                tc.tile_pool(name="ubp", bufs=2) as ubp,
                tc.tile_pool(name="abp", bufs=2) as abp,
                tc.tile_pool(name="outp", bufs=2) as outp,
            ):
                ys, us, abcs, vts = {}, {}, {}, {}

                def load_vis(b, h):
                    vt = visb.tile([128, ND * HLEN], FP16, tag="v", name="vt")
                    nc.scalar.dma_start(out=vt[:], in_=vis16[b, h])
                    vts[(b, h)] = vt

                def phase_a(b, h):
                    if h == 0:
                        us[b] = ubp.tile([128, Lv], F32, tag="u", name="u_t")
                        ys[b] = ubp.tile([128, Lv], FP16, tag="y", name="y_r")
                        # broadcast decay gate a=(1-delta); the scan consumes
                        # the same [128, 512] columns every chunk.
                        a_bc = abp.tile([128, 512], F32, tag="a", name="a_bc")
                        nc.gpsimd.memset(a_bc[:], 1.0)
                        nc.vector.tensor_scalar_mul(
                            a_bc[:], a_bc[:], a_sb[:, b : b + 1]
                        )
                        abcs[b] = a_bc
                    u_t, y_r, a_bc = us[b], ys[b], abcs[b]
                    vt = vts[(b, h)]
                    for i in range(NTH):
                        t = h * NTH + i
                        sl = slice(t * 512, (t + 1) * 512)
                        xp = psx.tile([128, 512], F32, tag="x", name="xp")
                        for d in range(ND):
                            for lo in (0, DS):  # col-tiled twins, concurrent
                                nc.tensor.matmul(
                                    xp[lo : lo + DS, :],
                                    wxt_sb[:, d, :],
                                    vt[:, d * HLEN + i * 512 : d * HLEN + (i + 1) * 512],
                                    start=(d == 0),
                                    stop=(d == ND - 1),
                                )
                        # u = (C*deltaB) * x_raw + (C*deltaB)*bx
                        nc.scalar.activation(
                            u_t[:, sl],
                            xp[:],
                            AF.Identity,
                            bias=cdbx_sb[:, b : b + 1],
                            scale=cdb_sb[:, b : b + 1],
                        )
                        # chunked scan; chain via the previous chunk's last col
                        nc.vector.tensor_tensor_scan(
                            y_r[:, sl],
                            a_bc[:],
                            u_t[:, sl],
                            0.0 if t == 0 else y_r[:, t * 512 - 1 : t * 512],
                            OP.mult,
                            OP.add,
                        )

                def evac(ps, dst, mc):
                    # PSUM evacuation (f32->fp16 + bias bo), split 2:1
                    # scalar:vector per measured engine rates.
                    if evac_i[0] % 3 < 2:
                        nc.scalar.activation(
                            dst, ps[:], AF.Identity,
                            bias=bo_sb[:, mc : mc + 1], scale=1.0,
                        )
                    else:
                        nc.vector.tensor_scalar_add(dst, ps[:], bo_sb[:, mc : mc + 1])
                    evac_i[0] += 1

                def phase_b(b, h):
                    y_r = ys[b]
                    for mcp in range(NMC // 2):
                        mcA, mcB = 2 * mcp, 2 * mcp + 1
                        otA = outp.tile([128, HLEN], FP16, tag="oA", name="otA")
                        otB = outp.tile([128, HLEN], FP16, tag="oB", name="otB")
                        for q in range(HLEN // 1024):
                            base = h * HLEN + q * 1024
                            psA = pso.tile([128, 1024], F32, tag="op", name="psA")
                            psB = pso.tile([128, 1024], F32, tag="op", name="psB")
                            for hh in range(2):
                                sl = slice(base + hh * 512, base + (hh + 1) * 512)
                                osl = slice(hh * 512, (hh + 1) * 512)
                                # row-packed pair: (0,0) and (64,0) tiles run
                                # concurrently; each LDW overlaps the other
                                # row-group's running matmul.
                                nc.tensor.matmul(
                                    psA[:, osl],
                                    wott_sb[0:DS, mcA * 128 : (mcA + 1) * 128],
                                    y_r[0:DS, sl],
                                    start=True,
                                    stop=True,
                                )
                                nc.tensor.matmul(
                                    psB[:, osl],
                                    wott_sb[DS:128, mcB * 128 : (mcB + 1) * 128],
                                    y_r[DS:128, sl],
                                    start=True,
                                    stop=True,
                                )
                            evac(psA, otA[:, q * 1024 : (q + 1) * 1024], mcA)
                            evac(psB, otB[:, q * 1024 : (q + 1) * 1024], mcB)
                        for mc, ot in ((mcA, otA), (mcB, otB)):
                            nc.sync.dma_start(
                                out=outT16[
                                    b,
                                    mc * 128 : (mc + 1) * 128,
                                    h * HLEN : (h + 1) * HLEN,
                                ],
                                in_=ot[:],
                            )

                load_vis(0, 0)
                load_vis(0, 1)
                phase_a(0, 0)
                load_vis(1, 0)
                phase_a(0, 1)
                phase_b(0, 0)
                load_vis(1, 1)
                phase_a(1, 0)
                phase_b(0, 1)
                phase_a(1, 1)
                phase_b(1, 0)
                phase_b(1, 1)
    return nc


def _prep_host_inputs(
    visual_feats, text_embeds, Wx, bx, Wd, bd, WB, bB, WC, bC, Wo, bo
):
    f = lambda a: np.asarray(a, dtype=np.float32)
    # [B, Lv, Dv] -> [B, NH, 128p, ND*HLEN] fp16 with element
    # (b, h, p, d*HLEN+t) = visual[b, h*HLEN+t, d*128+p]
    vis16 = np.ascontiguousarray(
        f(visual_feats)
        .transpose(0, 2, 1)
        .reshape(B, ND, 128, NH, HLEN)
        .transpose(0, 3, 2, 1, 4)
        .reshape(B, NH, 128, ND * HLEN)
        .astype(np.float16)
    )
    # [B, Lt, Dt] -> [B, 128p, NJ, Lt] with Dt index = j*128 + p
    text16 = np.ascontiguousarray(
        f(text_embeds)
        .transpose(0, 2, 1)
        .reshape(B, NJ, 128, Lt)
        .transpose(0, 2, 1, 3)
        .astype(np.float16)
    )
    # Wx.T [Dv, ds] -> [128p, ND, ds] with Dv index = c*128 + p
    wxt16 = np.ascontiguousarray(
        f(Wx).T.reshape(ND, 128, DS).transpose(1, 0, 2).astype(np.float16)
    )
    # Gate weights transposed, pre-scaled by 1/Lt (the text mean), fp16,
    # packed [Dt, 3, ds] -> [128p, NJ, 3, ds] with Dt index = j*128 + p.
    wg16 = np.ascontiguousarray(
        (np.stack([f(Wd).T, f(WB).T, f(WC).T], axis=1) / np.float32(Lt))
        .reshape(NJ, 128, 3, DS)
        .transpose(1, 0, 2, 3)
        .astype(np.float16)
    )
    wot = f(Wo).T.astype(np.float16)  # [ds, dm]
    wott16 = np.ascontiguousarray(np.concatenate([wot, wot], axis=0))  # [128, dm]
    # bo -> [128p, NMC] with dm index = mc*128 + p
    bo_t = np.ascontiguousarray(f(bo).reshape(NMC, 128).T)
    dup = lambda a: np.ascontiguousarray(
        np.concatenate([f(a).reshape(-1, 1)] * 2, axis=0)
    )
    shared = {
        "wxt16": wxt16,
        "wg16": wg16,
        "wott16": wott16,
        "bo_t": bo_t,
        "bd_c": dup(bd),
        "nbd_c": dup(-f(bd)),
        "bb_c": dup(bB),
        "bc_c": dup(bC),
        "bx_c": dup(bx),
    }
    in_maps = []
    for c in range(NCORES):
        m = dict(shared)
        m["vis16"] = np.ascontiguousarray(vis16[c * BPC : (c + 1) * BPC])
        m["text16"] = np.ascontiguousarray(text16[c * BPC : (c + 1) * BPC])
        in_maps.append(m)
    return in_maps


_PROGRAM_CACHE = {}


def _get_program(mm_dtype=MM_DTYPE):
    key = str(mm_dtype)
    if key not in _PROGRAM_CACHE:
        nc = _build_program(mm_dtype)
        if not nc.is_finalized():
            nc.finalize()
        _PROGRAM_CACHE[key] = nc
    return _PROGRAM_CACHE[key]


def run(inputs: dict, trace: bool = False, mm_dtype=MM_DTYPE):
    """Run the kernel on all 8 cores; returns (full_output, BassKernelResults)."""
    nc = _get_program(mm_dtype)
    in_maps = _prep_host_inputs(**inputs)
    res = run_bass_kernel_spmd(nc, in_maps, list(range(NCORES)), trace=trace)
    # outT16 is [BPC, DM, Lv] fp16 per core; un-transpose + upcast on host.
    full = np.concatenate(
        [
            np.swapaxes(res.results[i]["outT16"], 1, 2).astype(np.float32)
            for i in range(NCORES)
        ],
        axis=0,
    )
    return np.ascontiguousarray(full), res


def kernel(**inputs) -> np.ndarray:
    out, _ = run(inputs, trace=False)
    return out
